# revision 2
# baseline (speedup 1.0000x reference)
"""Cosine-similarity attention (B=8, H=16, N=1024, D=64) on 8 trn2 NeuronCores.

Sharding: core c gets batch c (all 16 heads). No cross-core communication.

Per-core algorithm (per head pair A,B):
  - load q,k rows as [128p, 2h, 8blk, 64d] fp32
  - row sum-of-squares (DVE), 1/norm = exp(-0.5*ln(ss)) (ACT, one table set
    shared with the main exp)
  - qn = q * rnorm (GpSimd, bf16 out), same for k
  - PE block-transpose qn/kn -> qnT/knT [128(2x64d), 1024] bf16 (A rows 0:64,
    B rows 64:128) via PSUM + DVE evac
  - row-tiled QK^T: logitsT[m,i] for both heads concurrently (K=64 halves of
    the PE array), into a [128, 2048] PSUM window per m-chunk
  - exp via ACT straight out of PSUM into bf16 SBUF (cosine logits are in
    [-1,1]: no max subtraction needed)
  - AV: out[i,:]|Z accumulated over m-chunks as expT.T @ [v|ones]
  - normalize by 1/Z (DVE) into a staged [128, 8, 16, 64] output, one DMA out
"""

import numpy as np
from contextlib import ExitStack

import concourse.bass as bass
import concourse.bacc as bacc
import concourse.mybir as mybir
import concourse.tile as tile
from concourse import bass_utils
from concourse.masks import make_identity

FP32 = mybir.dt.float32
BF16 = mybir.dt.bfloat16

N_CORES = 8
H = 16          # heads per core (= all heads; batch is sharded)
N = 1024
D = 64
NB = N // 128   # 8 row-blocks / m-chunks / i-blocks


def emit_attention(ctx: ExitStack, tc: tile.TileContext, q, k, v, out, n_heads=H):
    nc = tc.nc
    mult = mybir.AluOpType.mult
    AX = mybir.AxisListType.X
    Exp = mybir.ActivationFunctionType.Exp
    Ln = mybir.ActivationFunctionType.Ln

    singles = ctx.enter_context(tc.tile_pool(name="singles", bufs=1))
    qk_stage = ctx.enter_context(tc.tile_pool(name="qk_stage", bufs=2))
    v_stage = ctx.enter_context(tc.tile_pool(name="v_stage", bufs=2))
    small = ctx.enter_context(tc.tile_pool(name="small", bufs=2))
    qn_pool = ctx.enter_context(tc.tile_pool(name="qn_pool", bufs=2))
    tpose = ctx.enter_context(tc.tile_pool(name="tpose", bufs=2))
    expp = ctx.enter_context(tc.tile_pool(name="expp", bufs=2))
    zr_pool = ctx.enter_context(tc.tile_pool(name="zr_pool", bufs=4))

    tp_psum = ctx.enter_context(tc.tile_pool(name="tp_psum", bufs=2, space="PSUM"))
    qk_psum = ctx.enter_context(tc.tile_pool(name="qk_psum", bufs=1, space="PSUM"))
    av_psum = ctx.enter_context(tc.tile_pool(name="av_psum", bufs=2, space="PSUM"))

    identity = singles.tile([128, 128], BF16)
    make_identity(nc, identity)

    # out[128b + p, 64h + d] <- out_stage[p, b, h, d]
    out_stage = singles.tile([128, NB, n_heads, D], FP32)

    n_pairs = n_heads // 2
    for pair in range(n_pairs):
        hA = 2 * pair
        # ---- load raw q/k rows: partition p <- row 128*blk + p
        qraw = qk_stage.tile([128, 2, NB, D], FP32, tag="qraw")
        kraw = qk_stage.tile([128, 2, NB, D], FP32, tag="kraw")
        vraw = v_stage.tile([128, 2, NB, D], FP32, tag="vraw")
        src = lambda t: t[hA : hA + 2].rearrange("h (a p) d -> p h a d", p=128)
        nc.sync.dma_start(out=qraw, in_=src(q))
        nc.sync.dma_start(out=kraw, in_=src(k))
        nc.sync.dma_start(out=vraw, in_=src(v))

        # ---- row 1/norms: ss = sum_d x^2 ; rr = exp(-0.5 ln(ss)) = 1/||x||
        sq = small.tile([128, NB, D], FP32, tag="sq")
        ss = small.tile([128, 4, NB], FP32, tag="ss")
        for ih in range(2):
            nc.vector.tensor_tensor(sq, qraw[:, ih], qraw[:, ih], op=mult)
            nc.vector.reduce_sum(ss[:, ih], sq, axis=AX)
            nc.vector.tensor_tensor(sq, kraw[:, ih], kraw[:, ih], op=mult)
            nc.vector.reduce_sum(ss[:, 2 + ih], sq, axis=AX)
        lns = small.tile([128, 4, NB], FP32, tag="lns")
        rr = small.tile([128, 4, NB], FP32, tag="rr")
        nc.scalar.activation(lns, ss, Ln)
        nc.scalar.activation(rr, lns, Exp, scale=-0.5)

        # ---- normalize to bf16 (GpSimd so DVE stays free)
        qn = qn_pool.tile([128, 2, NB, D], BF16, tag="qn")
        kn = qn_pool.tile([128, 2, NB, D], BF16, tag="kn")
        for ih in range(2):
            for a in range(NB):
                nc.gpsimd.tensor_scalar_mul(qn[:, ih, a], qraw[:, ih, a], rr[:, ih, a : a + 1])
                nc.gpsimd.tensor_scalar_mul(kn[:, ih, a], kraw[:, ih, a], rr[:, 2 + ih, a : a + 1])

        # ---- v|ones in bf16: [128, 2, NB, 65]
        vb = v_stage.tile([128, 2, NB, D + 1], BF16, tag="vb")
        nc.gpsimd.memset(vb[:, :, :, D : D + 1], 1.0)
        nc.vector.tensor_copy(vb[:, :, :, 0:D], vraw)

        # ---- transpose qn/kn -> qnT/knT [128(=2x64d), 1024] bf16
        qnT = tpose.tile([128, N], BF16, tag="qnT")
        knT = tpose.tile([128, N], BF16, tag="knT")
        for srcT, dstT in ((qn, qnT), (kn, knT)):
            for a in range(NB):
                tp = tp_psum.tile([128, 128], BF16, tag="tp")
                nc.tensor.transpose(tp[0:64, :], srcT[:, 0, a], identity)
                nc.tensor.transpose(tp[64:128, :], srcT[:, 1, a], identity)
                nc.vector.tensor_copy(dstT[:, a * 128 : (a + 1) * 128], tp)

        # ---- QK^T (row-tiled, both heads) + exp, per m-chunk
        # eAB free layout: (m-chunk, head, i)
        eAB = expp.tile([128, NB, 2, N], BF16, tag="eAB")
        for mc in range(NB):
            win = qk_psum.tile([128, 2048], FP32, tag="win")
            for ih in range(2):
                for icc in range(2):
                    nc.tensor.matmul(
                        win[:, ih * 1024 + icc * 512 : ih * 1024 + (icc + 1) * 512],
                        lhsT=knT[ih * 64 : (ih + 1) * 64, mc * 128 : (mc + 1) * 128],
                        rhs=qnT[ih * 64 : (ih + 1) * 64, icc * 512 : (icc + 1) * 512],
                        start=True,
                        stop=True,
                        tile_position=(ih * 64, 0),
                    )
            nc.scalar.activation(eAB[:, mc], win, Exp)

        # ---- AV + normalize, per head / i-block
        for ih in range(2):
            for b in range(NB):
                acc = av_psum.tile([128, D + 1], FP32, tag="acc")
                for mc in range(NB):
                    nc.tensor.matmul(
                        acc,
                        lhsT=eAB[:, mc, ih, b * 128 : (b + 1) * 128],
                        rhs=vb[:, ih, mc, :],
                        start=(mc == 0),
                        stop=(mc == NB - 1),
                    )
                zr = zr_pool.tile([128, 1], FP32, tag="zr")
                nc.vector.reciprocal(zr, acc[:, D : D + 1])
                nc.vector.tensor_scalar_mul(out_stage[:, b, hA + ih], acc[:, 0:D], zr)

    nc.sync.dma_start(
        out=out.rearrange("(a p) (h d) -> p a h d", p=128, d=D),
        in_=out_stage,
    )


def build_program(n_heads=H, num_devices=N_CORES):
    nc = bacc.Bacc(
        "TRN2",
        target_bir_lowering=False,
        debug=False,
        enable_asserts=False,
        num_devices=num_devices,
    )
    qd = nc.dram_tensor("q", [n_heads, N, D], FP32, kind="ExternalInput").ap()
    kd = nc.dram_tensor("k", [n_heads, N, D], FP32, kind="ExternalInput").ap()
    vd = nc.dram_tensor("v", [n_heads, N, D], FP32, kind="ExternalInput").ap()
    od = nc.dram_tensor("out", [N, n_heads * D], FP32, kind="ExternalOutput").ap()
    with tile.TileContext(nc) as tc:
        with ExitStack() as ctx:
            emit_attention(ctx, tc, qd, kd, vd, od, n_heads=n_heads)
    nc.compile()
    return nc


_PROGRAM = None


def kernel(q: np.ndarray, k: np.ndarray, v: np.ndarray, _trace=False, _trace_kwargs=None):
    """Full inputs [8, 16, 1024, 64] fp32 -> full output [8, 1024, 1024] fp32."""
    global _PROGRAM
    if _PROGRAM is None:
        _PROGRAM = build_program()
    nc = _PROGRAM

    from concourse.bass_interp import get_hw_module

    in_maps = [
        {
            "q": np.ascontiguousarray(np.asarray(q)[c], dtype=np.float32),
            "k": np.ascontiguousarray(np.asarray(k)[c], dtype=np.float32),
            "v": np.ascontiguousarray(np.asarray(v)[c], dtype=np.float32),
        }
        for c in range(N_CORES)
    ]
    old_m = nc.m
    nc.m = get_hw_module(nc.m)
    try:
        res = bass_utils.run_bass_kernel_spmd(
            nc,
            in_maps,
            core_ids=list(range(N_CORES)),
            trace=_trace,
            **(_trace_kwargs or {}),
        )
    finally:
        nc.m = old_m
    out = np.stack([res.results[c]["out"] for c in range(N_CORES)])
    if _trace:
        kernel.last_results = res
    return out


# revision 6
# speedup vs baseline: 3164.8527x; 3164.8527x over previous
"""Cosine-similarity attention (B=8, H=16, N=1024, D=64) on 8 trn2 NeuronCores.

Sharding: core c gets batch c (all 16 heads). No cross-core communication.

Per-core algorithm (per head pair A,B):
  - load q,k rows as [128p, 2h, 8blk, 64d] fp32
  - row sum-of-squares (DVE), 1/norm = exp(-0.5*ln(ss)) (ACT, one table set
    shared with the main exp)
  - qn = q * rnorm (GpSimd, bf16 out), same for k
  - PE block-transpose qn/kn -> qnT/knT [128(2x64d), 1024] bf16 (A rows 0:64,
    B rows 64:128) via PSUM + DVE evac
  - row-tiled QK^T: logitsT[m,i] for both heads concurrently (K=64 halves of
    the PE array), into a [128, 2048] PSUM window per m-chunk
  - exp via ACT straight out of PSUM into bf16 SBUF (cosine logits are in
    [-1,1]: no max subtraction needed)
  - AV: out[i,:]|Z accumulated over m-chunks as expT.T @ [v|ones]
  - normalize by 1/Z (DVE) into a staged [128, 8, 16, 64] output, one DMA out
"""

import numpy as np
from contextlib import ExitStack

import concourse.bass as bass
import concourse.bacc as bacc
import concourse.mybir as mybir
import concourse.tile as tile
from concourse import bass_utils
from concourse.masks import make_identity

FP32 = mybir.dt.float32
BF16 = mybir.dt.bfloat16

N_CORES = 8
H = 16          # heads per core (= all heads; batch is sharded)
N = 1024
D = 64
NB = N // 128   # 8 row-blocks / m-chunks / i-blocks


def emit_attention(ctx: ExitStack, tc: tile.TileContext, q, k, v, out, n_heads=H):
    nc = tc.nc
    mult = mybir.AluOpType.mult
    AX = mybir.AxisListType.X
    Exp = mybir.ActivationFunctionType.Exp
    Ln = mybir.ActivationFunctionType.Ln

    singles = ctx.enter_context(tc.tile_pool(name="singles", bufs=1))
    qk_stage = ctx.enter_context(tc.tile_pool(name="qk_stage", bufs=2))
    v_stage = ctx.enter_context(tc.tile_pool(name="v_stage", bufs=2))
    small = ctx.enter_context(tc.tile_pool(name="small", bufs=2))
    qn_pool = ctx.enter_context(tc.tile_pool(name="qn_pool", bufs=2))
    tpose = ctx.enter_context(tc.tile_pool(name="tpose", bufs=2))
    expp = ctx.enter_context(tc.tile_pool(name="expp", bufs=2))
    zr_pool = ctx.enter_context(tc.tile_pool(name="zr_pool", bufs=4))

    tp_psum = ctx.enter_context(tc.tile_pool(name="tp_psum", bufs=2, space="PSUM"))
    qk_psum = ctx.enter_context(tc.tile_pool(name="qk_psum", bufs=1, space="PSUM"))
    av_psum = ctx.enter_context(tc.tile_pool(name="av_psum", bufs=2, space="PSUM"))

    identity = singles.tile([128, 128], BF16)
    make_identity(nc, identity)

    # out[128b + p, 64h + d] <- out_stage[p, b, h, d]
    out_stage = singles.tile([128, NB, n_heads, D], FP32)

    n_pairs = n_heads // 2
    for pair in range(n_pairs):
        hA = 2 * pair
        # ---- load raw q/k rows: partition p <- row 128*blk + p
        qraw = qk_stage.tile([128, 2, NB, D], FP32, tag="qraw")
        kraw = qk_stage.tile([128, 2, NB, D], FP32, tag="kraw")
        vraw = v_stage.tile([128, 2, NB, D], FP32, tag="vraw")
        # packed layout: partition p holds rows 8p..8p+7 (contiguous 2KB DMA
        # runs); tile index a = row-within-partition. All downstream APs stay
        # regular: transpose of sub-tile a yields columns/rows {8j+a}, and the
        # same permutation is applied consistently to v (m order is free under
        # softmax) and undone by the output DMA below.
        src = lambda t: t[hA : hA + 2].rearrange("h (p a) d -> p h a d", a=NB)
        nc.sync.dma_start(out=qraw, in_=src(q))
        nc.sync.dma_start(out=kraw, in_=src(k))
        nc.sync.dma_start(out=vraw, in_=src(v))

        # ---- row 1/norms: ss = sum_d x^2 ; rr = exp(-0.5 ln(ss)) = 1/||x||
        sq = small.tile([128, NB, D], FP32, tag="sq")
        ss = small.tile([128, 4, NB], FP32, tag="ss")
        for ih in range(2):
            nc.vector.tensor_tensor(sq, qraw[:, ih], qraw[:, ih], op=mult)
            nc.vector.reduce_sum(ss[:, ih], sq, axis=AX)
            nc.vector.tensor_tensor(sq, kraw[:, ih], kraw[:, ih], op=mult)
            nc.vector.reduce_sum(ss[:, 2 + ih], sq, axis=AX)
        lns = small.tile([128, 4, NB], FP32, tag="lns")
        rr = small.tile([128, 4, NB], FP32, tag="rr")
        nc.scalar.activation(lns, ss, Ln)
        nc.scalar.activation(rr, lns, Exp, scale=-0.5)

        # ---- normalize to bf16 (GpSimd so DVE stays free)
        qn = qn_pool.tile([128, 2, NB, D], BF16, tag="qn")
        kn = qn_pool.tile([128, 2, NB, D], BF16, tag="kn")
        for ih in range(2):
            for a in range(NB):
                nc.gpsimd.tensor_scalar_mul(qn[:, ih, a], qraw[:, ih, a], rr[:, ih, a : a + 1])
                nc.gpsimd.tensor_scalar_mul(kn[:, ih, a], kraw[:, ih, a], rr[:, 2 + ih, a : a + 1])

        # ---- v|ones in bf16: [128, 2, NB, 65]
        vb = v_stage.tile([128, 2, NB, D + 1], BF16, tag="vb")
        nc.gpsimd.memset(vb[:, :, :, D : D + 1], 1.0)
        nc.vector.tensor_copy(vb[:, :, :, 0:D], vraw)

        # ---- transpose qn/kn -> qnT/knT [128(=2x64d), 1024] bf16
        qnT = tpose.tile([128, N], BF16, tag="qnT")
        knT = tpose.tile([128, N], BF16, tag="knT")
        for srcT, dstT in ((qn, qnT), (kn, knT)):
            for a in range(NB):
                tp = tp_psum.tile([128, 128], BF16, tag="tp")
                nc.tensor.transpose(tp[0:64, :], srcT[:, 0, a], identity)
                nc.tensor.transpose(tp[64:128, :], srcT[:, 1, a], identity)
                nc.vector.tensor_copy(dstT[:, a * 128 : (a + 1) * 128], tp)

        # ---- QK^T (row-tiled, both heads) + exp, per m-chunk
        # eAB free layout: (m-chunk, head, i)
        eAB = expp.tile([128, NB, 2, N], BF16, tag="eAB")
        for mc in range(NB):
            win = qk_psum.tile([128, 2048], FP32, tag="win")
            for ih in range(2):
                for icc in range(2):
                    nc.tensor.matmul(
                        win[:, ih * 1024 + icc * 512 : ih * 1024 + (icc + 1) * 512],
                        lhsT=knT[ih * 64 : (ih + 1) * 64, mc * 128 : (mc + 1) * 128],
                        rhs=qnT[ih * 64 : (ih + 1) * 64, icc * 512 : (icc + 1) * 512],
                        start=True,
                        stop=True,
                        tile_position=(ih * 64, 0),
                    )
            nc.scalar.activation(eAB[:, mc], win, Exp)

        # ---- AV + normalize, per head / i-block
        for ih in range(2):
            for b in range(NB):
                acc = av_psum.tile([128, D + 1], FP32, tag="acc")
                for mc in range(NB):
                    nc.tensor.matmul(
                        acc,
                        lhsT=eAB[:, mc, ih, b * 128 : (b + 1) * 128],
                        rhs=vb[:, ih, mc, :],
                        start=(mc == 0),
                        stop=(mc == NB - 1),
                    )
                zr = zr_pool.tile([128, 1], FP32, tag="zr")
                nc.vector.reciprocal(zr, acc[:, D : D + 1])
                nc.vector.tensor_scalar_mul(out_stage[:, b, hA + ih], acc[:, 0:D], zr)

    nc.sync.dma_start(
        out=out.rearrange("(p a) (h d) -> p a h d", a=NB, d=D),
        in_=out_stage,
    )


class _Bacc(bacc.Bacc):
    """Bacc whose act-table pass only sees the combined ln+exp set, so Ln and
    Exp activations share one table load instead of thrashing between the
    single-function sets (~2.7us per reload on ACT)."""

    def insert_act_table_loads(self):
        import bass_rust as _bass_rust
        from concourse.hw_specs import get_activation_tables

        has_activation = any(
            isinstance(i, mybir.InstActivation)
            for b in self.main_func.blocks
            for i in b.instructions
        )
        if not has_activation:
            return
        tables = [
            (name, set() if name in ("exp_and_others", "natural_log", "exp_and_friends") else fns)
            for name, fns in get_activation_tables(self.m.arch).items()
        ]
        _bass_rust.insert_act_table_loads(self, tables)


def build_program(n_heads=H, num_devices=N_CORES, loop_iters=1):
    nc = _Bacc(
        "TRN2",
        target_bir_lowering=False,
        debug=False,
        enable_asserts=False,
        num_devices=num_devices,
    )
    qd = nc.dram_tensor("q", [n_heads, N, D], FP32, kind="ExternalInput").ap()
    kd = nc.dram_tensor("k", [n_heads, N, D], FP32, kind="ExternalInput").ap()
    vd = nc.dram_tensor("v", [n_heads, N, D], FP32, kind="ExternalInput").ap()
    od = nc.dram_tensor("out", [N, n_heads * D], FP32, kind="ExternalOutput").ap()
    with tile.TileContext(nc) as tc:
        with ExitStack() as ctx:
            if loop_iters > 1:
                with tc.For_i(0, loop_iters, 1):
                    with ExitStack() as ictx:
                        emit_attention(ictx, tc, qd, kd, vd, od, n_heads=n_heads)
            else:
                emit_attention(ctx, tc, qd, kd, vd, od, n_heads=n_heads)
    nc.compile()
    return nc


_PROGRAM = None


def kernel(q: np.ndarray, k: np.ndarray, v: np.ndarray, _trace=False, _trace_kwargs=None):
    """Full inputs [8, 16, 1024, 64] fp32 -> full output [8, 1024, 1024] fp32."""
    global _PROGRAM
    if _PROGRAM is None:
        _PROGRAM = build_program()
    nc = _PROGRAM

    from concourse.bass_interp import get_hw_module

    in_maps = [
        {
            "q": np.ascontiguousarray(np.asarray(q)[c], dtype=np.float32),
            "k": np.ascontiguousarray(np.asarray(k)[c], dtype=np.float32),
            "v": np.ascontiguousarray(np.asarray(v)[c], dtype=np.float32),
        }
        for c in range(N_CORES)
    ]
    old_m = nc.m
    nc.m = get_hw_module(nc.m)
    try:
        res = bass_utils.run_bass_kernel_spmd(
            nc,
            in_maps,
            core_ids=list(range(N_CORES)),
            trace=_trace,
            **(_trace_kwargs or {}),
        )
    finally:
        nc.m = old_m
    out = np.stack([res.results[c]["out"] for c in range(N_CORES)])
    if _trace:
        kernel.last_results = res
    return out


# revision 10
# speedup vs baseline: 7599.2757x; 2.4011x over previous
"""Cosine-similarity attention (B=8, H=16, N=1024, D=64) on 8 trn2 NeuronCores.

Sharding: core c gets batch c (all 16 heads). No cross-core communication.

Per-core algorithm (per head pair A,B):
  - load q,k rows as [128p, 2h, 8blk, 64d] fp32
  - row sum-of-squares (DVE), 1/norm = exp(-0.5*ln(ss)) (ACT, one table set
    shared with the main exp)
  - qn = q * rnorm (GpSimd, bf16 out), same for k
  - PE block-transpose qn/kn -> qnT/knT [128(2x64d), 1024] bf16 (A rows 0:64,
    B rows 64:128) via PSUM + DVE evac
  - row-tiled QK^T: logitsT[m,i] for both heads concurrently (K=64 halves of
    the PE array), into a [128, 2048] PSUM window per m-chunk
  - exp via ACT straight out of PSUM into bf16 SBUF (cosine logits are in
    [-1,1]: no max subtraction needed)
  - AV: out[i,:]|Z accumulated over m-chunks as expT.T @ [v|ones]
  - normalize by 1/Z (DVE) into a staged [128, 8, 16, 64] output, one DMA out
"""

import numpy as np
from contextlib import ExitStack

import concourse.bass as bass
import concourse.bacc as bacc
import concourse.mybir as mybir
import concourse.tile as tile
from concourse import bass_utils
from concourse.masks import make_identity

FP32 = mybir.dt.float32
BF16 = mybir.dt.bfloat16

N_CORES = 8
H = 16          # heads per core (= all heads; batch is sharded)
N = 1024
D = 64
NB = N // 128   # 8 row-blocks / m-chunks / i-blocks


def emit_attention(ctx: ExitStack, tc: tile.TileContext, q, k, v, out, n_heads=H):
    nc = tc.nc
    mult = mybir.AluOpType.mult
    AX = mybir.AxisListType.X
    Exp = mybir.ActivationFunctionType.Exp
    Ln = mybir.ActivationFunctionType.Ln

    def bcast(ap, n):
        # broadcast a [..., G] AP over a new innermost axis of length n
        return bass.AP(tensor=ap.tensor, offset=ap.offset, ap=[*ap.ap, [0, n]])

    singles = ctx.enter_context(tc.tile_pool(name="singles", bufs=1))
    qk_stage = ctx.enter_context(tc.tile_pool(name="qk_stage", bufs=2))
    v_stage = ctx.enter_context(tc.tile_pool(name="v_stage", bufs=2))
    small = ctx.enter_context(tc.tile_pool(name="small", bufs=2))
    qn_pool = ctx.enter_context(tc.tile_pool(name="qn_pool", bufs=2))
    tpose = ctx.enter_context(tc.tile_pool(name="tpose", bufs=2))
    expp = ctx.enter_context(tc.tile_pool(name="expp", bufs=2))
    zr_pool = ctx.enter_context(tc.tile_pool(name="zr_pool", bufs=4))
    out_pool = ctx.enter_context(tc.tile_pool(name="out_pool", bufs=2))

    tp_psum = ctx.enter_context(tc.tile_pool(name="tp_psum", bufs=2, space="PSUM"))
    qk_psum = ctx.enter_context(tc.tile_pool(name="qk_psum", bufs=1, space="PSUM"))
    av_psum = ctx.enter_context(tc.tile_pool(name="av_psum", bufs=2, space="PSUM"))

    identity = singles.tile([128, 128], BF16)
    make_identity(nc, identity)

    out_r = out.rearrange("(p a) (h d) -> p a h d", a=NB, d=D)

    n_pairs = n_heads // 2
    for pair in range(n_pairs):
        hA = 2 * pair
        # ---- load raw q/k/v. Packed layout: partition p holds rows
        # 8p..8p+7 (contiguous 2KB DMA runs); index a = row-within-partition.
        # The induced row permutation is consistent for q/k/v and undone by
        # the output DMA (softmax is invariant to m order; i order is a pure
        # row permutation of the output).
        # qk_raw index t (3rd dim): 0,1 = q headA/B ; 2,3 = k headA/B
        # (t inner of a so the per-a pair slice [:, a, t0:t0+2, :] is contiguous)
        qk_raw = qk_stage.tile([128, NB, 4, D], FP32, tag="qk_raw")
        vraw = v_stage.tile([128, 2, NB, D], FP32, tag="vraw")
        for ih in range(2):
            nc.sync.dma_start(
                out=qk_raw[:, :, ih],
                in_=q[hA + ih].rearrange("(p a) d -> p a d", a=NB),
            )
            nc.sync.dma_start(
                out=qk_raw[:, :, 2 + ih],
                in_=k[hA + ih].rearrange("(p a) d -> p a d", a=NB),
            )
            nc.sync.dma_start(
                out=vraw[:, ih],
                in_=v[hA + ih].rearrange("(p a) d -> p a d", a=NB),
            )

        # ---- row 1/norms: ss = sum_d x^2 ; rr = exp(-0.5 ln(ss)) = 1/||x||
        sq = small.tile([128, NB, 4, D], FP32, tag="sq")
        ss = small.tile([128, NB, 4], FP32, tag="ss")
        nc.vector.tensor_tensor(sq, qk_raw, qk_raw, op=mult)
        nc.vector.reduce_sum(ss, sq, axis=AX)
        lns = small.tile([128, NB, 4], FP32, tag="lns")
        rr = small.tile([128, NB, 4], FP32, tag="rr")
        nc.scalar.activation(lns, ss, Ln)
        nc.scalar.activation(rr, lns, Exp, scale=-0.5)

        # ---- normalize to bf16, one broadcast multiply (GpSimd keeps DVE free)
        qn_all = qn_pool.tile([128, NB, 4, D], BF16, tag="qn_all")
        nc.gpsimd.tensor_tensor(qn_all, qk_raw, bcast(rr, D), op=mult)

        # ---- v|ones in bf16: [128, 2, NB, 65]
        vb = v_stage.tile([128, 2, NB, D + 1], BF16, tag="vb")
        nc.gpsimd.memset(vb[:, :, :, D : D + 1], 1.0)
        nc.vector.tensor_copy(vb[:, :, :, 0:D], vraw)

        # ---- transpose -> qnT/knT [128(=A|B stacked 64d), 1024] bf16.
        # One PE transpose per a-block handles both heads: the strided
        # [128, 2, 64] slice streams against the same 128-col identity.
        qnT = tpose.tile([128, N], BF16, tag="qnT")
        knT = tpose.tile([128, N], BF16, tag="knT")
        for t0, dstT in ((0, qnT), (2, knT)):
            for ag in range(0, NB, 4):
                tp = tp_psum.tile([128, 4, 128], BF16, tag="tp")
                for j in range(4):
                    nc.tensor.transpose(
                        tp[:, j], qn_all[:, ag + j, t0 : t0 + 2, :], identity
                    )
                nc.vector.tensor_copy(dstT[:, ag * 128 : (ag + 4) * 128], tp)

        # ---- QK^T (row-tiled, both heads) + exp, per m-chunk
        # eAB free layout: (m-chunk, head, i)
        eAB = expp.tile([128, NB, 2, N], BF16, tag="eAB")
        for mc in range(NB):
            win = qk_psum.tile([128, 2048], FP32, tag="win")
            for ih in range(2):
                for icc in range(2):
                    nc.tensor.matmul(
                        win[:, ih * 1024 + icc * 512 : ih * 1024 + (icc + 1) * 512],
                        lhsT=knT[ih * 64 : (ih + 1) * 64, mc * 128 : (mc + 1) * 128],
                        rhs=qnT[ih * 64 : (ih + 1) * 64, icc * 512 : (icc + 1) * 512],
                        start=True,
                        stop=True,
                        tile_position=(ih * 64, 0),
                    )
            nc.scalar.activation(eAB[:, mc], win, Exp)

        # ---- AV + normalize (batched per 4 i-blocks), per head
        out_pair = out_pool.tile([128, NB, 2, D], FP32, tag="out_pair")
        for ih in range(2):
            for bg in range(0, NB, 4):
                acc = av_psum.tile([128, 4, D + 1], FP32, tag="acc")
                for j in range(4):
                    for mc in range(NB):
                        nc.tensor.matmul(
                            acc[:, j],
                            lhsT=eAB[:, mc, ih, (bg + j) * 128 : (bg + j + 1) * 128],
                            rhs=vb[:, ih, mc, :],
                            start=(mc == 0),
                            stop=(mc == NB - 1),
                        )
                zr = zr_pool.tile([128, 4], FP32, tag="zr")
                nc.vector.reciprocal(zr, acc[:, :, D])
                nc.vector.tensor_tensor(
                    out_pair[:, bg : bg + 4, ih],
                    acc[:, :, 0:D],
                    bcast(zr, D),
                    op=mult,
                )
        nc.sync.dma_start(out=out_r[:, :, hA : hA + 2, :], in_=out_pair)


class _Bacc(bacc.Bacc):
    """Bacc whose act-table pass only sees the combined ln+exp set, so Ln and
    Exp activations share one table load instead of thrashing between the
    single-function sets (~2.7us per reload on ACT)."""

    def insert_act_table_loads(self):
        import bass_rust as _bass_rust
        from concourse.hw_specs import get_activation_tables

        has_activation = any(
            isinstance(i, mybir.InstActivation)
            for b in self.main_func.blocks
            for i in b.instructions
        )
        if not has_activation:
            return
        tables = [
            (name, set() if name in ("exp_and_others", "natural_log", "exp_and_friends") else fns)
            for name, fns in get_activation_tables(self.m.arch).items()
        ]
        _bass_rust.insert_act_table_loads(self, tables)


def build_program(n_heads=H, num_devices=N_CORES, loop_iters=1):
    nc = _Bacc(
        "TRN2",
        target_bir_lowering=False,
        debug=False,
        enable_asserts=False,
        num_devices=num_devices,
    )
    qd = nc.dram_tensor("q", [n_heads, N, D], FP32, kind="ExternalInput").ap()
    kd = nc.dram_tensor("k", [n_heads, N, D], FP32, kind="ExternalInput").ap()
    vd = nc.dram_tensor("v", [n_heads, N, D], FP32, kind="ExternalInput").ap()
    od = nc.dram_tensor("out", [N, n_heads * D], FP32, kind="ExternalOutput").ap()
    with tile.TileContext(nc) as tc:
        with ExitStack() as ctx:
            if loop_iters > 1:
                with tc.For_i(0, loop_iters, 1):
                    with ExitStack() as ictx:
                        emit_attention(ictx, tc, qd, kd, vd, od, n_heads=n_heads)
            else:
                emit_attention(ctx, tc, qd, kd, vd, od, n_heads=n_heads)
    nc.compile()
    return nc


_PROGRAM = None


def kernel(q: np.ndarray, k: np.ndarray, v: np.ndarray, _trace=False, _trace_kwargs=None):
    """Full inputs [8, 16, 1024, 64] fp32 -> full output [8, 1024, 1024] fp32."""
    global _PROGRAM
    if _PROGRAM is None:
        _PROGRAM = build_program()
    nc = _PROGRAM

    from concourse.bass_interp import get_hw_module

    in_maps = [
        {
            "q": np.ascontiguousarray(np.asarray(q)[c], dtype=np.float32),
            "k": np.ascontiguousarray(np.asarray(k)[c], dtype=np.float32),
            "v": np.ascontiguousarray(np.asarray(v)[c], dtype=np.float32),
        }
        for c in range(N_CORES)
    ]
    old_m = nc.m
    nc.m = get_hw_module(nc.m)
    try:
        res = bass_utils.run_bass_kernel_spmd(
            nc,
            in_maps,
            core_ids=list(range(N_CORES)),
            trace=_trace,
            **(_trace_kwargs or {}),
        )
    finally:
        nc.m = old_m
    out = np.stack([res.results[c]["out"] for c in range(N_CORES)])
    if _trace:
        kernel.last_results = res
    return out
